# revision 33
# baseline (speedup 1.0000x reference)
"""Trainium2 Bass kernel for nn_EntmaxAttention.

Computation (per batch row b of B=64):
    h      = tanh(X_b @ W1 + b1)            X_b: (S=2048, H=1024), W1: (H, A=64)
    scores = h @ W2 + b2                    -> (S,)
    w      = entmax15(scores)               sparse attention weights (S,)
    ctx    = w @ X_b                        -> (H,)
Returns (context (B, H), weights (B, S)).

Sharding: data-parallel over batch, 8 rows per NeuronCore, no collectives.

Per-core pipeline, one batch row at a time (X_b = 8 MB stays in SBUF):
  DMA X_b -> PE-transpose 128x128 tiles of X_b into X^T slices -> MLP matmul
  (float32r, full fp32 data) -> tanh (ACT, bias fused) -> scores matmul ->
  entmax-1.5 via Newton iterations on the dual threshold tau (DVE only) ->
  weighted-sum matmul (w stationary, X natural-layout moving) -> DMA out.

entmax15(z) = [max(z/2 - max(z/2) - tau, 0)]^2 with tau s.t. the squares
sum to 1. g(tau) = sum(relu(x - tau)^2) - 1 is convex and decreasing; Newton
from tau = -1 (guaranteed g >= 0) converges monotonically; 8 iters suffice
for fp32, we run 10. This matches the reference's sort-based tau to ~1e-6.
"""

import os

import numpy as np

import concourse.bass as bass
import concourse.tile as tile
from concourse import bacc, mybir
from concourse.bass_utils import run_bass_kernel_spmd

B, S, H, A = 64, 2048, 1024, 64
NCORES = 8
BPC = B // NCORES           # batch rows per core
ST = S // 128               # 16 s-tiles of 128
KT = H // 128               # 8 contraction tiles for the MLP matmul
NEWTON_ITERS = 10

F32 = mybir.dt.float32
F32R = mybir.dt.float32r
AF = mybir.ActivationFunctionType
OP = mybir.AluOpType


def _emit(nc, tc, ctx, aps, bpc):
    x_d, w1_d, b1_d, w2_d, ctx_d, wout_d, ident_d = aps

    consts = ctx.enter_context(tc.tile_pool(name="consts", bufs=1))
    xpool = ctx.enter_context(tc.tile_pool(name="x", bufs=2))
    bpool = ctx.enter_context(tc.tile_pool(name="bounce", bufs=2))
    xtpool = ctx.enter_context(tc.tile_pool(name="xt", bufs=2))
    htpool = ctx.enter_context(tc.tile_pool(name="ht", bufs=1))
    wpool = ctx.enter_context(tc.tile_pool(name="w", bufs=2))
    scr = ctx.enter_context(tc.tile_pool(name="scr", bufs=1))
    cspool = ctx.enter_context(tc.tile_pool(name="cs", bufs=2))
    # PSUM: 3 banks transpose staging (shared with small tiles), 4 banks MLP
    # accumulators, 1 bank persistent weighted-sum accumulator = 8.
    ptpool = ctx.enter_context(tc.tile_pool(name="ptpsum", bufs=3, space="PSUM"))
    mpsum = ctx.enter_context(tc.tile_pool(name="mpsum", bufs=4, space="PSUM"))
    cpsum = ctx.enter_context(tc.tile_pool(name="cpsum", bufs=1, space="PSUM"))

    # Constants. fp32r matmul operands must be produced by a rounding engine
    # op (never a DMA), so weights/identity pass through one-time copies.
    # fp32r matmuls additionally require the full 128-column PE array
    # (col_grp == 0xf) and even element counts, so narrow stationaries are
    # zero-padded to 128 columns (the extra output rows cost no cycles).
    w1_raw = bpool.tile([128, KT * A], F32, tag="bounce")
    nc.sync.dma_start(
        w1_raw[:].rearrange("p (k a) -> p k a", k=KT),
        w1_d.ap().rearrange("(k p) a -> p k a", p=128))
    zc128 = consts.tile([128, 2], F32)
    nc.vector.memset(zc128[:], 0.0)
    w1pad = consts.tile([128, KT * 128], F32R)
    nc.vector.tensor_copy(
        w1pad[:], zc128[:, 0:1].to_broadcast((128, KT * 128)))
    nc.scalar.copy(
        w1pad[:].rearrange("p (k m) -> p k m", k=KT)[:, :, 0:A],
        w1_raw[:].rearrange("p (k a) -> p k a", k=KT))
    b1_sb = consts.tile([A, 1], F32)
    nc.sync.dma_start(b1_sb[:], b1_d.ap().unsqueeze(1))
    w2_raw = consts.tile([A, 1], F32)
    nc.sync.dma_start(w2_raw[:], w2_d.ap())
    w2pad = consts.tile([A, 2], F32R)
    nc.vector.tensor_copy(w2pad[:], zc128[0:A, 0:1].to_broadcast((A, 2)))
    nc.scalar.copy(w2pad[:, 0:1], w2_raw[:])
    ident_raw = consts.tile([128, 128], F32)
    nc.sync.dma_start(ident_raw[:], ident_d.ap())
    ident_a = consts.tile([128, 128], F32)
    nc.scalar.copy(ident_a[:], ident_raw[:])
    ident_r = consts.tile([128, 128], F32R)
    nc.gpsimd.tensor_copy(ident_r[:], ident_raw[:])
    zcst = consts.tile([ST, 128], F32)
    nc.vector.memset(zcst[:], 0.0)
    # weighted-sum stationary: per s-tile a [128, 128] block whose column 0
    # holds the entmax weights, the rest zeros (keeps col_grp == 0xf)
    wpad = consts.tile([128, ST * 128], F32R)
    nc.vector.tensor_copy(
        wpad[:], zc128[:, 0:1].to_broadcast((128, ST * 128)))
    sc_sb = consts.tile([128, 128], F32)
    nc.vector.memset(sc_sb[:], 0.0)
    # persistent weighted-sum PSUM accumulator (one bank, all batches)
    ctxps = cpsum.tile([128, 512], F32)

    stage = int(os.environ.get("KERNEL_STAGE", "5"))

    def phase1(b):
        """DMA + MLP + scores + entmax for batch row b. Returns (x_sb, w_sb)."""
        # X_b as [128, 16*1024] fp32r: column block j holds s in [128j, 128j+128).
        # DMA raw fp32 into bounce chunks; GpSimd rounds into x_sb (single
        # writer, keeps every consumer at <= 2 sync waits).
        x_sb = xpool.tile([128, ST * H], F32R, tag="x")
        for ch in range(ST):
            bounce = bpool.tile([128, H], F32, tag="bounce")
            nc.sync.dma_start(
                bounce[:], x_d.ap()[b, ch * 128:(ch + 1) * 128, :])
            nc.gpsimd.tensor_copy(x_sb[:, ch * H:(ch + 1) * H], bounce[:])

        if stage < 2:
            return x_sb, None
        # MLP matmul: hT[a, s] = sum_h W1[h, a] * X[s, h], via PE-transposed
        # X^T slices (fp32r transpose, 1.5 cy/row).
        hps = [mpsum.tile([128, 512], F32, tag="mp", name=f"hps{b}_{i}")
               for i in range(4)]
        for hc in range(KT):
            xt = xtpool.tile([128, S], F32R, tag="xt")
            for jg in range(4):
                pt = ptpool.tile([128, 512], F32R, tag="tp")
                for q in range(4):
                    j = jg * 4 + q
                    nc.tensor.transpose(
                        pt[:, q * 128:(q + 1) * 128],
                        x_sb[:, j * H + hc * 128: j * H + (hc + 1) * 128],
                        ident_r[:])
                nc.scalar.copy(xt[:, jg * 512:(jg + 1) * 512], pt[:])
            for sc in range(4):
                nc.tensor.matmul(
                    hps[sc][:],
                    w1pad[:, hc * 128:(hc + 1) * 128],
                    xt[:, sc * 512:(sc + 1) * 512],
                    start=(hc == 0), stop=(hc == KT - 1))

        # tanh (bias fused); hT [64, 2048] fp32r
        ht = htpool.tile([A, S], F32R, tag="ht")
        for sc in range(4):
            nc.scalar.activation(
                ht[:, sc * 512:(sc + 1) * 512], hps[sc][0:A, :], AF.Tanh,
                bias=b1_sb[:], scale=1.0)

        if stage < 3:
            return x_sb, None
        # scores[s] = sum_a hT[a, s] * W2[a]  -> even columns of psum [128, 32]
        scps = ptpool.tile([128, 2 * ST], F32, tag="tp")
        for j in range(ST):
            nc.tensor.matmul(
                scps[:, 2 * j:2 * j + 2],
                ht[:, j * 128:(j + 1) * 128],
                w2pad[:],
                start=True, stop=True)
        # sc_sb is a zero-initialized [128, 128] so the score transpose can
        # run full-width (col_grp 0xf); only columns 0:16 are live.
        nc.scalar.copy(
            sc_sb[:, 0:ST],
            scps[:].rearrange("p (j two) -> p j two", two=2)[:, :, 0:1])
        stps = ptpool.tile([128, 128], F32, tag="tp")
        nc.tensor.transpose(stps[:], sc_sb[:], ident_a[:])
        sch = scr.tile([ST, 128], F32, tag="sch")
        nc.scalar.mul(sch[:], stps[0:ST, :], 0.5)

        if stage < 4:
            return x_sb, None
        # entmax-1.5 threshold via Newton, all on DVE. Cross-partition
        # reduce/broadcast go through the 32x32 StreamTranspose; all APs
        # are base-partition-0.
        sc32 = scr.tile([32, 64], F32, tag="sc32")
        scT = scr.tile([32, 64], F32, tag="scT")
        bc32 = scr.tile([32, 32], F32, tag="bc32")
        bcT = scr.tile([32, 32], F32, tag="bcT")
        nc.vector.memset(sc32[:], 0.0)
        nc.vector.memset(bc32[:], 0.0)
        nc.vector.tensor_reduce(
            sc32[0:ST, 0:1], sch[:], mybir.AxisListType.X, OP.max)
        nc.vector.transpose(scT[:, 0:32], sc32[:, 0:32])
        mval = scr.tile([1, 1], F32, tag="mval")
        nc.vector.tensor_reduce(
            mval[:], scT[0:1, 0:ST], mybir.AxisListType.X, OP.max)
        nc.vector.tensor_copy(
            bc32[0:1, 0:ST], mval[0:1, 0:1].to_broadcast((1, ST)))
        nc.vector.transpose(bcT[:], bc32[:])
        z = scr.tile([ST, 128], F32, tag="z")
        nc.vector.tensor_scalar(z[:], sch[:], bcT[0:ST, 0:1], None,
                                OP.subtract)

        tb32 = scr.tile([32, 32], F32, tag="tb32")
        tbT = scr.tile([32, 32], F32, tag="tbT")
        nc.vector.memset(tb32[:], 0.0)
        nc.vector.memset(tbT[:], -1.0)
        r = scr.tile([ST, 128], F32, tag="r")
        r2 = scr.tile([ST, 128], F32, tag="r2")
        uu = scr.tile([1, 2], F32, tag="uu")
        urec = scr.tile([1, 1], F32, tag="urec")
        tau0 = scr.tile([1, 1], F32, tag="tau0")
        nc.vector.memset(tau0[:], -1.0)
        for it in range(NEWTON_ITERS):
            nc.vector.scalar_tensor_tensor(
                r[:], z[:], tbT[0:ST, 0:1], zcst[:],
                OP.subtract, OP.max, accum_out=sc32[0:ST, 0:1])
            nc.vector.scalar_tensor_tensor(
                r2[:], r[:], 1.0, r[:], OP.mult, OP.mult,
                accum_out=sc32[0:ST, 32:33])
            nc.vector.transpose(scT[:], sc32[:])
            nc.vector.tensor_reduce(
                uu[:], scT[0:1, :].rearrange("p (g f) -> p g f", g=2),
                mybir.AxisListType.X, OP.add)
            nc.vector.reciprocal(urec[:], uu[:, 0:1])
            nc.vector.scalar_tensor_tensor(
                urec[:], uu[:, 1:2], -1.0, urec[:], OP.add, OP.mult)
            nc.vector.scalar_tensor_tensor(
                tau0[:], urec[:], 0.5, tau0[:], OP.mult, OP.add)
            nc.vector.tensor_copy(
                tb32[0:1, 0:ST], tau0[0:1, 0:1].to_broadcast((1, ST)))
            nc.vector.transpose(tbT[:], tb32[:])

        # weights w = relu(z - tau)^2 in [16, 128] layout, rounded to fp32r
        w16 = wpool.tile([ST, 128], F32R, tag="w16")
        nc.vector.scalar_tensor_tensor(
            r[:], z[:], tbT[0:ST, 0:1], zcst[:], OP.subtract, OP.max)
        nc.vector.tensor_tensor(w16[:], r[:], r[:], OP.mult)
        nc.sync.dma_start(
            wout_d.ap()[b, :].rearrange("(j f) -> j f", j=ST).bitcast(F32R),
            w16[:])
        # transpose to [128, 16] for the weighted-sum stationary operand
        wtps = ptpool.tile([128, ST], F32R, tag="tp")
        nc.tensor.transpose(wtps[:], w16[:], ident_r[0:ST, 0:ST])
        return x_sb, wtps

    def phase2(b, x_sb, wtps):
        """ctx[h] = sum_s w[s] * X[s, h]; two h-passes through one PSUM bank.
        The stationary per s-tile is a [128, 128] block with the weights in
        column 0 and zeros elsewhere (fp32r needs the full PE array), so the
        context lands in psum row 0 and rows 1:128 accumulate zeros."""
        nc.scalar.copy(
            wpad[:].rearrange("p (j m) -> p j m", j=ST)[:, :, 0:1],
            wtps[:].unsqueeze(2))
        cs = cspool.tile([1, H], F32, tag="cs")
        for hp in range(2):
            for j in range(ST):
                nc.tensor.matmul(
                    ctxps[:],
                    wpad[:, j * 128:(j + 1) * 128],
                    x_sb[:, j * H + hp * 512: j * H + (hp + 1) * 512],
                    start=(j == 0), stop=(j == ST - 1))
            nc.scalar.copy(cs[:, hp * 512:(hp + 1) * 512], ctxps[0:1, :])
        nc.sync.dma_start(ctx_d.ap()[b, :].unsqueeze(0), cs[:])

    # software-pipelined emission: weighted sum of row b-1 overlaps row b
    live = None
    for b in range(bpc + 1):
        nxt = phase1(b) if b < bpc else None
        if live is not None and stage >= 5 and live[1] is not None:
            phase2(b - 1, *live)
        live = nxt


def build(bpc=BPC):
    from contextlib import ExitStack

    nc = bacc.Bacc("TRN2", target_bir_lowering=False, debug=False)
    x_d = nc.dram_tensor("x", [bpc, S, H], F32, kind="ExternalInput")
    w1_d = nc.dram_tensor("w1", [H, A], F32, kind="ExternalInput")
    b1_d = nc.dram_tensor("b1", [A], F32, kind="ExternalInput")
    w2_d = nc.dram_tensor("w2", [A, 1], F32, kind="ExternalInput")
    ctx_d = nc.dram_tensor("ctx", [bpc, H], F32, kind="ExternalOutput")
    wout_d = nc.dram_tensor("wout", [bpc, S], F32, kind="ExternalOutput")
    ident_d = nc.inline_tensor(np.eye(128, dtype=np.float32), name="ident")

    with tile.TileContext(nc) as tc:
        with ExitStack() as ctx:
            _emit(nc, tc, ctx, (x_d, w1_d, b1_d, w2_d, ctx_d, wout_d, ident_d),
                  bpc)
    nc.compile()
    return nc


_CACHE = {}


def kernel(lstm_output, W1, b1, W2, b2, _trace=False):
    lstm_output = np.ascontiguousarray(lstm_output, dtype=np.float32)
    if "nc" not in _CACHE:
        _CACHE["nc"] = build()
    nc = _CACHE["nc"]

    in_maps = []
    for c in range(NCORES):
        in_maps.append({
            "x": lstm_output[c * BPC:(c + 1) * BPC],
            "w1": np.ascontiguousarray(W1, dtype=np.float32),
            "b1": np.ascontiguousarray(b1, dtype=np.float32),
            "w2": np.ascontiguousarray(W2, dtype=np.float32),
        })
    res = run_bass_kernel_spmd(nc, in_maps, list(range(NCORES)), trace=_trace)
    context = np.concatenate([res.results[c]["ctx"] for c in range(NCORES)], 0)
    weights = np.concatenate([res.results[c]["wout"] for c in range(NCORES)], 0)
    if _trace:
        _CACHE["last_exec_time_ns"] = res.exec_time_ns
        _CACHE["last_results"] = res
    return context.astype(np.float32), weights.astype(np.float32)


# revision 34
# speedup vs baseline: 1.0770x; 1.0770x over previous
"""Trainium2 Bass kernel for nn_EntmaxAttention.

Computation (per batch row b of B=64):
    h      = tanh(X_b @ W1 + b1)            X_b: (S=2048, H=1024), W1: (H, A=64)
    scores = h @ W2 + b2                    -> (S,)
    w      = entmax15(scores)               sparse attention weights (S,)
    ctx    = w @ X_b                        -> (H,)
Returns (context (B, H), weights (B, S)).

Sharding: data-parallel over batch, 8 rows per NeuronCore, no collectives.

Per-core pipeline, one batch row at a time (X_b = 8 MB stays in SBUF):
  DMA X_b -> PE-transpose 128x128 tiles of X_b into X^T slices -> MLP matmul
  (float32r, full fp32 data) -> tanh (ACT, bias fused) -> scores matmul ->
  entmax-1.5 via Newton iterations on the dual threshold tau (DVE only) ->
  weighted-sum matmul (w stationary, X natural-layout moving) -> DMA out.

entmax15(z) = [max(z/2 - max(z/2) - tau, 0)]^2 with tau s.t. the squares
sum to 1. g(tau) = sum(relu(x - tau)^2) - 1 is convex and decreasing; Newton
from tau = -1 (guaranteed g >= 0) converges monotonically; 8 iters suffice
for fp32, we run 10. This matches the reference's sort-based tau to ~1e-6.
"""

import os

import numpy as np

import concourse.bass as bass
import concourse.tile as tile
from concourse import bacc, mybir
from concourse.bass_utils import run_bass_kernel_spmd

B, S, H, A = 64, 2048, 1024, 64
NCORES = 8
BPC = B // NCORES           # batch rows per core
ST = S // 128               # 16 s-tiles of 128
KT = H // 128               # 8 contraction tiles for the MLP matmul
NEWTON_ITERS = 10

F32 = mybir.dt.float32
F32R = mybir.dt.float32r
AF = mybir.ActivationFunctionType
OP = mybir.AluOpType


def _emit(nc, tc, ctx, aps, bpc):
    x_d, w1_d, b1_d, w2_d, ctx_d, wout_d, ident_d = aps

    consts = ctx.enter_context(tc.tile_pool(name="consts", bufs=1))
    xpool = ctx.enter_context(tc.tile_pool(name="x", bufs=2))
    bpool = ctx.enter_context(tc.tile_pool(name="bounce", bufs=2))
    xtpool = ctx.enter_context(tc.tile_pool(name="xt", bufs=2))
    htpool = ctx.enter_context(tc.tile_pool(name="ht", bufs=1))
    wpool = ctx.enter_context(tc.tile_pool(name="w", bufs=2))
    scr = ctx.enter_context(tc.tile_pool(name="scr", bufs=1))
    cspool = ctx.enter_context(tc.tile_pool(name="cs", bufs=2))
    # PSUM: 3 banks transpose staging (shared with small tiles), 4 banks MLP
    # accumulators, 1 bank persistent weighted-sum accumulator = 8.
    ptpool = ctx.enter_context(tc.tile_pool(name="ptpsum", bufs=3, space="PSUM"))
    mpsum = ctx.enter_context(tc.tile_pool(name="mpsum", bufs=4, space="PSUM"))
    cpsum = ctx.enter_context(tc.tile_pool(name="cpsum", bufs=1, space="PSUM"))

    # Constants. fp32r matmul operands must be produced by a rounding engine
    # op (never a DMA), so weights/identity pass through one-time copies.
    # fp32r matmuls additionally require the full 128-column PE array
    # (col_grp == 0xf) and even element counts, so narrow stationaries are
    # zero-padded to 128 columns (the extra output rows cost no cycles).
    w1_raw = bpool.tile([128, KT * A], F32, tag="bounce")
    nc.sync.dma_start(
        w1_raw[:].rearrange("p (k a) -> p k a", k=KT),
        w1_d.ap().rearrange("(k p) a -> p k a", p=128))
    zc128 = consts.tile([128, 2], F32)
    nc.vector.memset(zc128[:], 0.0)
    w1pad = consts.tile([128, KT * 128], F32R)
    nc.vector.tensor_copy(
        w1pad[:], zc128[:, 0:1].to_broadcast((128, KT * 128)))
    nc.scalar.copy(
        w1pad[:].rearrange("p (k m) -> p k m", k=KT)[:, :, 0:A],
        w1_raw[:].rearrange("p (k a) -> p k a", k=KT))
    b1_sb = consts.tile([A, 1], F32)
    nc.sync.dma_start(b1_sb[:], b1_d.ap().unsqueeze(1))
    w2_raw = consts.tile([A, 1], F32)
    nc.sync.dma_start(w2_raw[:], w2_d.ap())
    w2pad = consts.tile([A, 2], F32R)
    nc.vector.tensor_copy(w2pad[:], zc128[0:A, 0:1].to_broadcast((A, 2)))
    nc.scalar.copy(w2pad[:, 0:1], w2_raw[:])
    ident_raw = consts.tile([128, 128], F32)
    nc.sync.dma_start(ident_raw[:], ident_d.ap())
    ident_a = consts.tile([128, 128], F32)
    nc.scalar.copy(ident_a[:], ident_raw[:])
    ident_r = consts.tile([128, 128], F32R)
    nc.gpsimd.tensor_copy(ident_r[:], ident_raw[:])
    zcst = consts.tile([ST, 128], F32)
    nc.vector.memset(zcst[:], 0.0)
    # weighted-sum stationary: per s-tile a [128, 128] block whose column 0
    # holds the entmax weights, the rest zeros (keeps col_grp == 0xf)
    wpad = consts.tile([128, ST * 128], F32R)
    nc.vector.tensor_copy(
        wpad[:], zc128[:, 0:1].to_broadcast((128, ST * 128)))
    sc_sb = consts.tile([128, 128], F32)
    nc.vector.memset(sc_sb[:], 0.0)
    # persistent weighted-sum PSUM accumulator (one bank, all batches)
    ctxps = cpsum.tile([128, 512], F32)

    stage = int(os.environ.get("KERNEL_STAGE", "5"))

    def phase1(b):
        """DMA + MLP + scores + entmax for batch row b. Returns (x_sb, w_sb)."""
        # X_b as [128, 16*1024] fp32r: column block j holds s in [128j, 128j+128).
        # DMA raw fp32 into bounce chunks; GpSimd rounds into x_sb (single
        # writer, keeps every consumer at <= 2 sync waits).
        x_sb = xpool.tile([128, ST * H], F32R, tag="x")
        for ch in range(ST):
            bounce = bpool.tile([128, H], F32, tag="bounce")
            nc.sync.dma_start(
                bounce[:], x_d.ap()[b, ch * 128:(ch + 1) * 128, :])
            # split the rounding casts across POOL and DVE: POOL alone takes
            # ~56us/row and gates the PE transposes (the trace's per-row gap)
            eng = nc.gpsimd if ch % 2 == 0 else nc.vector
            eng.tensor_copy(x_sb[:, ch * H:(ch + 1) * H], bounce[:])

        if stage < 2:
            return x_sb, None
        # MLP matmul: hT[a, s] = sum_h W1[h, a] * X[s, h], via PE-transposed
        # X^T slices (fp32r transpose, 1.5 cy/row).
        hps = [mpsum.tile([128, 512], F32, tag="mp", name=f"hps{b}_{i}")
               for i in range(4)]
        for hc in range(KT):
            xt = xtpool.tile([128, S], F32R, tag="xt")
            for jg in range(4):
                pt = ptpool.tile([128, 512], F32R, tag="tp")
                for q in range(4):
                    j = jg * 4 + q
                    nc.tensor.transpose(
                        pt[:, q * 128:(q + 1) * 128],
                        x_sb[:, j * H + hc * 128: j * H + (hc + 1) * 128],
                        ident_r[:])
                nc.scalar.copy(xt[:, jg * 512:(jg + 1) * 512], pt[:])
            for sc in range(4):
                nc.tensor.matmul(
                    hps[sc][:],
                    w1pad[:, hc * 128:(hc + 1) * 128],
                    xt[:, sc * 512:(sc + 1) * 512],
                    start=(hc == 0), stop=(hc == KT - 1))

        # tanh (bias fused); hT [64, 2048] fp32r
        ht = htpool.tile([A, S], F32R, tag="ht")
        for sc in range(4):
            nc.scalar.activation(
                ht[:, sc * 512:(sc + 1) * 512], hps[sc][0:A, :], AF.Tanh,
                bias=b1_sb[:], scale=1.0)

        if stage < 3:
            return x_sb, None
        # scores[s] = sum_a hT[a, s] * W2[a]  -> even columns of psum [128, 32]
        scps = ptpool.tile([128, 2 * ST], F32, tag="tp")
        for j in range(ST):
            nc.tensor.matmul(
                scps[:, 2 * j:2 * j + 2],
                ht[:, j * 128:(j + 1) * 128],
                w2pad[:],
                start=True, stop=True)
        # sc_sb is a zero-initialized [128, 128] so the score transpose can
        # run full-width (col_grp 0xf); only columns 0:16 are live.
        nc.scalar.copy(
            sc_sb[:, 0:ST],
            scps[:].rearrange("p (j two) -> p j two", two=2)[:, :, 0:1])
        stps = ptpool.tile([128, 128], F32, tag="tp")
        nc.tensor.transpose(stps[:], sc_sb[:], ident_a[:])
        sch = scr.tile([ST, 128], F32, tag="sch")
        nc.scalar.mul(sch[:], stps[0:ST, :], 0.5)

        if stage < 4:
            return x_sb, None
        # entmax-1.5 threshold via Newton, all on DVE. Cross-partition
        # reduce/broadcast go through the 32x32 StreamTranspose; all APs
        # are base-partition-0.
        sc32 = scr.tile([32, 64], F32, tag="sc32")
        scT = scr.tile([32, 64], F32, tag="scT")
        bc32 = scr.tile([32, 32], F32, tag="bc32")
        bcT = scr.tile([32, 32], F32, tag="bcT")
        nc.vector.memset(sc32[:], 0.0)
        nc.vector.memset(bc32[:], 0.0)
        nc.vector.tensor_reduce(
            sc32[0:ST, 0:1], sch[:], mybir.AxisListType.X, OP.max)
        nc.vector.transpose(scT[:, 0:32], sc32[:, 0:32])
        mval = scr.tile([1, 1], F32, tag="mval")
        nc.vector.tensor_reduce(
            mval[:], scT[0:1, 0:ST], mybir.AxisListType.X, OP.max)
        nc.vector.tensor_copy(
            bc32[0:1, 0:ST], mval[0:1, 0:1].to_broadcast((1, ST)))
        nc.vector.transpose(bcT[:], bc32[:])
        z = scr.tile([ST, 128], F32, tag="z")
        nc.vector.tensor_scalar(z[:], sch[:], bcT[0:ST, 0:1], None,
                                OP.subtract)

        tb32 = scr.tile([32, 32], F32, tag="tb32")
        tbT = scr.tile([32, 32], F32, tag="tbT")
        nc.vector.memset(tb32[:], 0.0)
        nc.vector.memset(tbT[:], -1.0)
        r = scr.tile([ST, 128], F32, tag="r")
        r2 = scr.tile([ST, 128], F32, tag="r2")
        uu = scr.tile([1, 2], F32, tag="uu")
        urec = scr.tile([1, 1], F32, tag="urec")
        tau0 = scr.tile([1, 1], F32, tag="tau0")
        nc.vector.memset(tau0[:], -1.0)
        for it in range(NEWTON_ITERS):
            nc.vector.scalar_tensor_tensor(
                r[:], z[:], tbT[0:ST, 0:1], zcst[:],
                OP.subtract, OP.max, accum_out=sc32[0:ST, 0:1])
            nc.vector.scalar_tensor_tensor(
                r2[:], r[:], 1.0, r[:], OP.mult, OP.mult,
                accum_out=sc32[0:ST, 32:33])
            nc.vector.transpose(scT[:], sc32[:])
            nc.vector.tensor_reduce(
                uu[:], scT[0:1, :].rearrange("p (g f) -> p g f", g=2),
                mybir.AxisListType.X, OP.add)
            nc.vector.reciprocal(urec[:], uu[:, 0:1])
            nc.vector.scalar_tensor_tensor(
                urec[:], uu[:, 1:2], -1.0, urec[:], OP.add, OP.mult)
            nc.vector.scalar_tensor_tensor(
                tau0[:], urec[:], 0.5, tau0[:], OP.mult, OP.add)
            nc.vector.tensor_copy(
                tb32[0:1, 0:ST], tau0[0:1, 0:1].to_broadcast((1, ST)))
            nc.vector.transpose(tbT[:], tb32[:])

        # weights w = relu(z - tau)^2 in [16, 128] layout, rounded to fp32r
        w16 = wpool.tile([ST, 128], F32R, tag="w16")
        nc.vector.scalar_tensor_tensor(
            r[:], z[:], tbT[0:ST, 0:1], zcst[:], OP.subtract, OP.max)
        nc.vector.tensor_tensor(w16[:], r[:], r[:], OP.mult)
        nc.sync.dma_start(
            wout_d.ap()[b, :].rearrange("(j f) -> j f", j=ST).bitcast(F32R),
            w16[:])
        # transpose to [128, 16] for the weighted-sum stationary operand
        wtps = ptpool.tile([128, ST], F32R, tag="tp")
        nc.tensor.transpose(wtps[:], w16[:], ident_r[0:ST, 0:ST])
        return x_sb, wtps

    def phase2(b, x_sb, wtps):
        """ctx[h] = sum_s w[s] * X[s, h]; two h-passes through one PSUM bank.
        The stationary per s-tile is a [128, 128] block with the weights in
        column 0 and zeros elsewhere (fp32r needs the full PE array), so the
        context lands in psum row 0 and rows 1:128 accumulate zeros."""
        nc.scalar.copy(
            wpad[:].rearrange("p (j m) -> p j m", j=ST)[:, :, 0:1],
            wtps[:].unsqueeze(2))
        cs = cspool.tile([1, H], F32, tag="cs")
        for hp in range(2):
            for j in range(ST):
                nc.tensor.matmul(
                    ctxps[:],
                    wpad[:, j * 128:(j + 1) * 128],
                    x_sb[:, j * H + hp * 512: j * H + (hp + 1) * 512],
                    start=(j == 0), stop=(j == ST - 1))
            nc.scalar.copy(cs[:, hp * 512:(hp + 1) * 512], ctxps[0:1, :])
        nc.sync.dma_start(ctx_d.ap()[b, :].unsqueeze(0), cs[:])

    # software-pipelined emission: weighted sum of row b-1 overlaps row b
    live = None
    for b in range(bpc + 1):
        nxt = phase1(b) if b < bpc else None
        if live is not None and stage >= 5 and live[1] is not None:
            phase2(b - 1, *live)
        live = nxt


def build(bpc=BPC):
    from contextlib import ExitStack

    nc = bacc.Bacc("TRN2", target_bir_lowering=False, debug=False)
    x_d = nc.dram_tensor("x", [bpc, S, H], F32, kind="ExternalInput")
    w1_d = nc.dram_tensor("w1", [H, A], F32, kind="ExternalInput")
    b1_d = nc.dram_tensor("b1", [A], F32, kind="ExternalInput")
    w2_d = nc.dram_tensor("w2", [A, 1], F32, kind="ExternalInput")
    ctx_d = nc.dram_tensor("ctx", [bpc, H], F32, kind="ExternalOutput")
    wout_d = nc.dram_tensor("wout", [bpc, S], F32, kind="ExternalOutput")
    ident_d = nc.inline_tensor(np.eye(128, dtype=np.float32), name="ident")

    with tile.TileContext(nc) as tc:
        with ExitStack() as ctx:
            _emit(nc, tc, ctx, (x_d, w1_d, b1_d, w2_d, ctx_d, wout_d, ident_d),
                  bpc)
    nc.compile()
    return nc


_CACHE = {}


def kernel(lstm_output, W1, b1, W2, b2, _trace=False):
    lstm_output = np.ascontiguousarray(lstm_output, dtype=np.float32)
    if "nc" not in _CACHE:
        _CACHE["nc"] = build()
    nc = _CACHE["nc"]

    in_maps = []
    for c in range(NCORES):
        in_maps.append({
            "x": lstm_output[c * BPC:(c + 1) * BPC],
            "w1": np.ascontiguousarray(W1, dtype=np.float32),
            "b1": np.ascontiguousarray(b1, dtype=np.float32),
            "w2": np.ascontiguousarray(W2, dtype=np.float32),
        })
    res = run_bass_kernel_spmd(nc, in_maps, list(range(NCORES)), trace=_trace)
    context = np.concatenate([res.results[c]["ctx"] for c in range(NCORES)], 0)
    weights = np.concatenate([res.results[c]["wout"] for c in range(NCORES)], 0)
    if _trace:
        _CACHE["last_exec_time_ns"] = res.exec_time_ns
        _CACHE["last_results"] = res
    return context.astype(np.float32), weights.astype(np.float32)


# revision 35
# speedup vs baseline: 1.2246x; 1.1371x over previous
"""Trainium2 Bass kernel for nn_EntmaxAttention.

Computation (per batch row b of B=64):
    h      = tanh(X_b @ W1 + b1)            X_b: (S=2048, H=1024), W1: (H, A=64)
    scores = h @ W2 + b2                    -> (S,)
    w      = entmax15(scores)               sparse attention weights (S,)
    ctx    = w @ X_b                        -> (H,)
Returns (context (B, H), weights (B, S)).

Sharding: data-parallel over batch, 8 rows per NeuronCore, no collectives.

Per-core pipeline, one batch row at a time (X_b = 8 MB stays in SBUF):
  DMA X_b -> PE-transpose 128x128 tiles of X_b into X^T slices -> MLP matmul
  (float32r, full fp32 data) -> tanh (ACT, bias fused) -> scores matmul ->
  entmax-1.5 via Newton iterations on the dual threshold tau (DVE only) ->
  weighted-sum matmul (w stationary, X natural-layout moving) -> DMA out.

entmax15(z) = [max(z/2 - max(z/2) - tau, 0)]^2 with tau s.t. the squares
sum to 1. g(tau) = sum(relu(x - tau)^2) - 1 is convex and decreasing; Newton
from tau = -1 (guaranteed g >= 0) converges monotonically; 8 iters suffice
for fp32, we run 10. This matches the reference's sort-based tau to ~1e-6.
"""

import os

import numpy as np

import concourse.bass as bass
import concourse.tile as tile
from concourse import bacc, mybir
from concourse.bass_utils import run_bass_kernel_spmd

B, S, H, A = 64, 2048, 1024, 64
NCORES = 8
BPC = B // NCORES           # batch rows per core
ST = S // 128               # 16 s-tiles of 128
KT = H // 128               # 8 contraction tiles for the MLP matmul
NEWTON_ITERS = 10

F32 = mybir.dt.float32
F32R = mybir.dt.float32r
AF = mybir.ActivationFunctionType
OP = mybir.AluOpType


def _emit(nc, tc, ctx, aps, bpc):
    x_d, w1_d, b1_d, w2_d, ctx_d, wout_d, ident_d = aps

    consts = ctx.enter_context(tc.tile_pool(name="consts", bufs=1))
    xpool = ctx.enter_context(tc.tile_pool(name="x", bufs=2))
    bpool = ctx.enter_context(tc.tile_pool(name="bounce", bufs=2))
    xtpool = ctx.enter_context(tc.tile_pool(name="xt", bufs=2))
    htpool = ctx.enter_context(tc.tile_pool(name="ht", bufs=1))
    wpool = ctx.enter_context(tc.tile_pool(name="w", bufs=2))
    scr = ctx.enter_context(tc.tile_pool(name="scr", bufs=1))
    cspool = ctx.enter_context(tc.tile_pool(name="cs", bufs=2))
    # PSUM: 3 banks transpose staging (shared with small tiles), 4 banks MLP
    # accumulators, 1 bank persistent weighted-sum accumulator = 8.
    ptpool = ctx.enter_context(tc.tile_pool(name="ptpsum", bufs=3, space="PSUM"))
    mpsum = ctx.enter_context(tc.tile_pool(name="mpsum", bufs=4, space="PSUM"))
    cpsum = ctx.enter_context(tc.tile_pool(name="cpsum", bufs=1, space="PSUM"))

    # Constants. fp32r matmul operands must be produced by a rounding engine
    # op (never a DMA), so weights/identity pass through one-time copies.
    # fp32r matmuls additionally require the full 128-column PE array
    # (col_grp == 0xf) and even element counts, so narrow stationaries are
    # zero-padded to 128 columns (the extra output rows cost no cycles).
    w1_raw = bpool.tile([128, KT * A], F32, tag="bounce")
    nc.sync.dma_start(
        w1_raw[:].rearrange("p (k a) -> p k a", k=KT),
        w1_d.ap().rearrange("(k p) a -> p k a", p=128))
    zc128 = consts.tile([128, 2], F32)
    nc.vector.memset(zc128[:], 0.0)
    w1pad = consts.tile([128, KT * 128], F32R)
    nc.vector.tensor_copy(
        w1pad[:], zc128[:, 0:1].to_broadcast((128, KT * 128)))
    nc.scalar.copy(
        w1pad[:].rearrange("p (k m) -> p k m", k=KT)[:, :, 0:A],
        w1_raw[:].rearrange("p (k a) -> p k a", k=KT))
    b1_sb = consts.tile([A, 1], F32)
    nc.sync.dma_start(b1_sb[:], b1_d.ap().unsqueeze(1))
    w2_raw = consts.tile([A, 1], F32)
    nc.sync.dma_start(w2_raw[:], w2_d.ap())
    w2pad = consts.tile([A, 2], F32R)
    nc.vector.tensor_copy(w2pad[:], zc128[0:A, 0:1].to_broadcast((A, 2)))
    nc.scalar.copy(w2pad[:, 0:1], w2_raw[:])
    ident_raw = consts.tile([128, 128], F32)
    nc.sync.dma_start(ident_raw[:], ident_d.ap())
    ident_a = consts.tile([128, 128], F32)
    nc.scalar.copy(ident_a[:], ident_raw[:])
    ident_r = consts.tile([128, 128], F32R)
    nc.gpsimd.tensor_copy(ident_r[:], ident_raw[:])
    zcst = consts.tile([ST, 128], F32)
    nc.vector.memset(zcst[:], 0.0)
    # weighted-sum stationary: per s-tile a [128, 128] block whose column 0
    # holds the entmax weights, the rest zeros (keeps col_grp == 0xf)
    wpad = consts.tile([128, ST * 128], F32R)
    nc.vector.tensor_copy(
        wpad[:], zc128[:, 0:1].to_broadcast((128, ST * 128)))
    sc_sb = consts.tile([128, 128], F32)
    nc.vector.memset(sc_sb[:], 0.0)
    # persistent weighted-sum PSUM accumulator (one bank, all batches)
    ctxps = cpsum.tile([128, 512], F32)

    stage = int(os.environ.get("KERNEL_STAGE", "5"))

    def phase1(b):
        """DMA + MLP + scores + entmax for batch row b. Returns (x_sb, w_sb)."""
        # X_b as [128, 16*1024] fp32r: column block j holds s in [128j, 128j+128).
        # DMA raw fp32 into bounce chunks; GpSimd rounds into x_sb (single
        # writer, keeps every consumer at <= 2 sync waits).
        x_sb = xpool.tile([128, ST * H], F32R, tag="x")
        for ch in range(ST):
            bounce = bpool.tile([128, H], F32, tag="bounce")
            nc.sync.dma_start(
                bounce[:], x_d.ap()[b, ch * 128:(ch + 1) * 128, :])
            # split the rounding casts across POOL and DVE: POOL alone takes
            # ~56us/row and gates the PE transposes (the trace's per-row gap)
            eng = nc.gpsimd if ch % 4 == 0 else nc.vector
            eng.tensor_copy(x_sb[:, ch * H:(ch + 1) * H], bounce[:])

        if stage < 2:
            return x_sb, None
        # MLP matmul: hT[a, s] = sum_h W1[h, a] * X[s, h], via PE-transposed
        # X^T slices (fp32r transpose, 1.5 cy/row).
        hps = [mpsum.tile([128, 512], F32, tag="mp", name=f"hps{b}_{i}")
               for i in range(4)]
        for hc in range(KT):
            xt = xtpool.tile([128, S], F32R, tag="xt")
            for jg in range(4):
                pt = ptpool.tile([128, 512], F32R, tag="tp")
                for q in range(4):
                    j = jg * 4 + q
                    nc.tensor.transpose(
                        pt[:, q * 128:(q + 1) * 128],
                        x_sb[:, j * H + hc * 128: j * H + (hc + 1) * 128],
                        ident_r[:])
                nc.scalar.copy(xt[:, jg * 512:(jg + 1) * 512], pt[:])
            for sc in range(4):
                nc.tensor.matmul(
                    hps[sc][:],
                    w1pad[:, hc * 128:(hc + 1) * 128],
                    xt[:, sc * 512:(sc + 1) * 512],
                    start=(hc == 0), stop=(hc == KT - 1))

        # tanh (bias fused); hT [64, 2048] fp32r
        ht = htpool.tile([A, S], F32R, tag="ht")
        for sc in range(4):
            nc.scalar.activation(
                ht[:, sc * 512:(sc + 1) * 512], hps[sc][0:A, :], AF.Tanh,
                bias=b1_sb[:], scale=1.0)

        if stage < 3:
            return x_sb, None
        # scores[s] = sum_a hT[a, s] * W2[a]  -> even columns of psum [128, 32]
        scps = ptpool.tile([128, 2 * ST], F32, tag="tp")
        for j in range(ST):
            nc.tensor.matmul(
                scps[:, 2 * j:2 * j + 2],
                ht[:, j * 128:(j + 1) * 128],
                w2pad[:],
                start=True, stop=True)
        # sc_sb is a zero-initialized [128, 128] so the score transpose can
        # run full-width (col_grp 0xf); only columns 0:16 are live.
        nc.scalar.copy(
            sc_sb[:, 0:ST],
            scps[:].rearrange("p (j two) -> p j two", two=2)[:, :, 0:1])
        stps = ptpool.tile([128, 128], F32, tag="tp")
        nc.tensor.transpose(stps[:], sc_sb[:], ident_a[:])
        sch = scr.tile([ST, 128], F32, tag="sch")
        nc.scalar.mul(sch[:], stps[0:ST, :], 0.5)

        if stage < 4:
            return x_sb, None
        # entmax-1.5 threshold via Newton, all on DVE. Cross-partition
        # reduce/broadcast go through the 32x32 StreamTranspose; all APs
        # are base-partition-0.
        sc32 = scr.tile([32, 64], F32, tag="sc32")
        scT = scr.tile([32, 64], F32, tag="scT")
        bc32 = scr.tile([32, 32], F32, tag="bc32")
        bcT = scr.tile([32, 32], F32, tag="bcT")
        nc.vector.memset(sc32[:], 0.0)
        nc.vector.memset(bc32[:], 0.0)
        nc.vector.tensor_reduce(
            sc32[0:ST, 0:1], sch[:], mybir.AxisListType.X, OP.max)
        nc.vector.transpose(scT[:, 0:32], sc32[:, 0:32])
        mval = scr.tile([1, 1], F32, tag="mval")
        nc.vector.tensor_reduce(
            mval[:], scT[0:1, 0:ST], mybir.AxisListType.X, OP.max)
        nc.vector.tensor_copy(
            bc32[0:1, 0:ST], mval[0:1, 0:1].to_broadcast((1, ST)))
        nc.vector.transpose(bcT[:], bc32[:])
        z = scr.tile([ST, 128], F32, tag="z")
        nc.vector.tensor_scalar(z[:], sch[:], bcT[0:ST, 0:1], None,
                                OP.subtract)

        tb32 = scr.tile([32, 32], F32, tag="tb32")
        tbT = scr.tile([32, 32], F32, tag="tbT")
        nc.vector.memset(tb32[:], 0.0)
        nc.vector.memset(tbT[:], -1.0)
        r = scr.tile([ST, 128], F32, tag="r")
        r2 = scr.tile([ST, 128], F32, tag="r2")
        uu = scr.tile([1, 2], F32, tag="uu")
        urec = scr.tile([1, 1], F32, tag="urec")
        tau0 = scr.tile([1, 1], F32, tag="tau0")
        nc.vector.memset(tau0[:], -1.0)
        for it in range(NEWTON_ITERS):
            nc.vector.scalar_tensor_tensor(
                r[:], z[:], tbT[0:ST, 0:1], zcst[:],
                OP.subtract, OP.max, accum_out=sc32[0:ST, 0:1])
            nc.vector.scalar_tensor_tensor(
                r2[:], r[:], 1.0, r[:], OP.mult, OP.mult,
                accum_out=sc32[0:ST, 32:33])
            nc.vector.transpose(scT[:], sc32[:])
            nc.vector.tensor_reduce(
                uu[:], scT[0:1, :].rearrange("p (g f) -> p g f", g=2),
                mybir.AxisListType.X, OP.add)
            nc.vector.reciprocal(urec[:], uu[:, 0:1])
            nc.vector.scalar_tensor_tensor(
                urec[:], uu[:, 1:2], -1.0, urec[:], OP.add, OP.mult)
            nc.vector.scalar_tensor_tensor(
                tau0[:], urec[:], 0.5, tau0[:], OP.mult, OP.add)
            nc.vector.tensor_copy(
                tb32[0:1, 0:ST], tau0[0:1, 0:1].to_broadcast((1, ST)))
            nc.vector.transpose(tbT[:], tb32[:])

        # weights w = relu(z - tau)^2 in [16, 128] layout, rounded to fp32r
        w16 = wpool.tile([ST, 128], F32R, tag="w16")
        nc.vector.scalar_tensor_tensor(
            r[:], z[:], tbT[0:ST, 0:1], zcst[:], OP.subtract, OP.max)
        nc.vector.tensor_tensor(w16[:], r[:], r[:], OP.mult)
        nc.sync.dma_start(
            wout_d.ap()[b, :].rearrange("(j f) -> j f", j=ST).bitcast(F32R),
            w16[:])
        # transpose to [128, 16] for the weighted-sum stationary operand
        wtps = ptpool.tile([128, ST], F32R, tag="tp")
        nc.tensor.transpose(wtps[:], w16[:], ident_r[0:ST, 0:ST])
        return x_sb, wtps

    def phase2(b, x_sb, wtps):
        """ctx[h] = sum_s w[s] * X[s, h]; two h-passes through one PSUM bank.
        The stationary per s-tile is a [128, 128] block with the weights in
        column 0 and zeros elsewhere (fp32r needs the full PE array), so the
        context lands in psum row 0 and rows 1:128 accumulate zeros."""
        nc.scalar.copy(
            wpad[:].rearrange("p (j m) -> p j m", j=ST)[:, :, 0:1],
            wtps[:].unsqueeze(2))
        cs = cspool.tile([1, H], F32, tag="cs")
        for hp in range(2):
            for j in range(ST):
                nc.tensor.matmul(
                    ctxps[:],
                    wpad[:, j * 128:(j + 1) * 128],
                    x_sb[:, j * H + hp * 512: j * H + (hp + 1) * 512],
                    start=(j == 0), stop=(j == ST - 1))
            nc.scalar.copy(cs[:, hp * 512:(hp + 1) * 512], ctxps[0:1, :])
        nc.sync.dma_start(ctx_d.ap()[b, :].unsqueeze(0), cs[:])

    # software-pipelined emission: weighted sum of row b-1 overlaps row b
    live = None
    for b in range(bpc + 1):
        nxt = phase1(b) if b < bpc else None
        if live is not None and stage >= 5 and live[1] is not None:
            phase2(b - 1, *live)
        live = nxt


def build(bpc=BPC):
    from contextlib import ExitStack

    nc = bacc.Bacc("TRN2", target_bir_lowering=False, debug=False)
    x_d = nc.dram_tensor("x", [bpc, S, H], F32, kind="ExternalInput")
    w1_d = nc.dram_tensor("w1", [H, A], F32, kind="ExternalInput")
    b1_d = nc.dram_tensor("b1", [A], F32, kind="ExternalInput")
    w2_d = nc.dram_tensor("w2", [A, 1], F32, kind="ExternalInput")
    ctx_d = nc.dram_tensor("ctx", [bpc, H], F32, kind="ExternalOutput")
    wout_d = nc.dram_tensor("wout", [bpc, S], F32, kind="ExternalOutput")
    ident_d = nc.inline_tensor(np.eye(128, dtype=np.float32), name="ident")

    with tile.TileContext(nc) as tc:
        with ExitStack() as ctx:
            _emit(nc, tc, ctx, (x_d, w1_d, b1_d, w2_d, ctx_d, wout_d, ident_d),
                  bpc)
    nc.compile()
    return nc


_CACHE = {}


def kernel(lstm_output, W1, b1, W2, b2, _trace=False):
    lstm_output = np.ascontiguousarray(lstm_output, dtype=np.float32)
    if "nc" not in _CACHE:
        _CACHE["nc"] = build()
    nc = _CACHE["nc"]

    in_maps = []
    for c in range(NCORES):
        in_maps.append({
            "x": lstm_output[c * BPC:(c + 1) * BPC],
            "w1": np.ascontiguousarray(W1, dtype=np.float32),
            "b1": np.ascontiguousarray(b1, dtype=np.float32),
            "w2": np.ascontiguousarray(W2, dtype=np.float32),
        })
    res = run_bass_kernel_spmd(nc, in_maps, list(range(NCORES)), trace=_trace)
    context = np.concatenate([res.results[c]["ctx"] for c in range(NCORES)], 0)
    weights = np.concatenate([res.results[c]["wout"] for c in range(NCORES)], 0)
    if _trace:
        _CACHE["last_exec_time_ns"] = res.exec_time_ns
        _CACHE["last_results"] = res
    return context.astype(np.float32), weights.astype(np.float32)
